# revision 1
# baseline (speedup 1.0000x reference)
"""Bahdanau-style additive attention on 8 TRN2 NeuronCores.

Reference computation (B=32, S=2048, H=1024):
    query  = hidden @ Wq.T                      # (B, H)
    keys   = enc @ Wk.T                         # (B, S, H)
    energy = tanh(query[:, None, :] + keys)     # (B, S, H)
    attn   = energy @ v                         # (B, S)
    out    = softmax(mask(attn, lengths))       # (B, S)

Sharding: data-parallel, 4 batches per core, no collectives.

Per-core dataflow (all matmuls in fp32r mode — 1 cycle/row on the PE):
    - enc is fed pre-transposed (H, S) so the H contraction sits on SBUF
      partitions; tiles are [128h x 512s].
    - keys tile [128f, 512s] accumulates 8 h-chunk matmuls in PSUM.
    - ACT applies tanh with the per-partition query bias, PSUM -> SBUF.
    - PE contracts the energy tile with v ([128,1] stationary) into a
      [1, 512] PSUM accumulator over the 8 f-chunks.
    - DVE adds the (host-precomputed) length-mask bias and keeps a
      running per-batch max; the per-batch softmax tail is ACT exp with
      accumulate, then the normalize runs split across DVE and ACT.
"""

import sys

if "/opt/trn_rl_repo" not in sys.path:
    sys.path.insert(0, "/opt/trn_rl_repo")

import numpy as np

B, S, H = 32, 2048, 1024
NCORES = 8
BPC = B // NCORES  # batches per core
FT = 128           # partition tile (feature / h chunk)
HC = H // FT       # h chunks
ST = 512           # seq tile
NST = S // ST

_CACHE = {}


def _build(variant="full", loop_r=1):
    import concourse.bass as bass  # noqa: F401
    import concourse.tile as tile
    from concourse import bacc, mybir

    f32 = mybir.dt.float32
    f32r = mybir.dt.float32r
    Tanh = mybir.ActivationFunctionType.Tanh
    Exp = mybir.ActivationFunctionType.Exp

    nc = bacc.Bacc("TRN2", target_bir_lowering=False, debug=False,
                   num_devices=NCORES)

    encT = nc.dram_tensor("encT", [BPC, H, S], f32r, kind="ExternalInput").ap()
    hT = nc.dram_tensor("hT", [H, BPC], f32r, kind="ExternalInput").ap()
    wkT = nc.dram_tensor("wkT", [H, H], f32r, kind="ExternalInput").ap()
    wqT = nc.dram_tensor("wqT", [H, H], f32r, kind="ExternalInput").ap()
    vp = nc.dram_tensor("vp", [FT, HC], f32r, kind="ExternalInput").ap()
    # mask rows live at partition 32*b so engine APs stay 32-aligned
    mb = nc.dram_tensor("mb", [FT, S], f32, kind="ExternalInput").ap()
    out = nc.dram_tensor("out", [BPC, S], f32, kind="ExternalOutput").ap()

    nraw = 3 if loop_r == 1 else 0

    with tile.TileContext(nc) as tc:
        with (
            tc.tile_pool(name="singles", bufs=1) as singles,
            tc.tile_pool(name="encp", bufs=4) as encp,
            tc.tile_pool(name="energy", bufs=4) as ep,
            tc.tile_pool(name="kpsum", bufs=5, space="PSUM") as kps,
            tc.tile_pool(name="apsum", bufs=3, space="PSUM") as aps,
            tc.tile_pool(name="stats", bufs=1) as stats,
        ):
            wk_sb = singles.tile([FT, HC, H], f32r)
            wq_sb = singles.tile([FT, HC, H], f32r)
            ht_sb = singles.tile([FT, HC, BPC], f32r)
            v_sb = singles.tile([FT, HC], f32r)
            mask_sb = singles.tile([FT, S], f32)
            qT_sb = singles.tile([FT, HC, BPC], f32)
            attn_sb = singles.tile([FT, S], f32)
            nm_sb = singles.tile([FT, 1], f32)
            nc.vector.memset(attn_sb[:], 0.0)
            nc.vector.memset(nm_sb[:], -3.0e38)
            enr = (singles.tile([FT, nraw * HC, ST], f32r, name="enr")
                   if nraw else None)

            def emit_q():
                # qT[f, b] = sum_h WqT[h, f] * hiddenT[h, b].  All 8 fc
                # accumulation groups share one PSUM tile: `start` only
                # on the global first matmul (per-element has_written
                # handles first-touch for the other regions; nothing
                # reads the tile until after the global stop).
                qp = aps.tile([FT, HC, BPC], f32, tag="ap")
                for hc in range(HC):
                    for fc in range(HC):
                        nc.tensor.matmul(
                            qp[:, fc, :],
                            lhsT=wq_sb[:, hc, fc * FT:(fc + 1) * FT],
                            rhs=ht_sb[:, hc, :],
                            start=(hc == 0 and fc == 0),
                            stop=(hc == HC - 1 and fc == HC - 1),
                            skip_group_check=True)
                nc.vector.tensor_copy(out=qT_sb[:], in_=qp[:])

            def load_wq():
                for hc in range(HC):
                    nc.sync.dma_start(out=wq_sb[:, hc, :],
                                      in_=wqT[hc * FT:(hc + 1) * FT, :])
                    nc.sync.dma_start(out=ht_sb[:, hc, :],
                                      in_=hT[hc * FT:(hc + 1) * FT, :])

            def load_et(b, st):
                et = encp.tile([FT, HC, ST], f32r, tag="et", name="et")
                for hc in range(HC):
                    nc.sync.dma_start(
                        out=et[:, hc, :],
                        in_=encT[b, hc * FT:(hc + 1) * FT,
                                 st * ST:(st + 1) * ST])
                return et

            def load_wk_col(fc):
                nc.sync.dma_start(
                    out=wk_sb[:, :, fc * FT:(fc + 1) * FT],
                    in_=wkT[:, fc * FT:(fc + 1) * FT].rearrange(
                        "(hc p) f -> p hc f", p=FT))

            raw_ets = []
            if nraw:
                # DMA order: Wk column 0 in per-hc 64KB blocks
                # interleaved with the et0 chunks (the first keys matmul
                # needs only block hc=0, so the PE starts at ~1.5us),
                # remaining Wk columns (keys group fc only needs col
                # fc), other raw tiles, Wq LAST — the PE has ~40us of
                # keys work queued by the time q runs.
                et0 = encp.tile([FT, HC, ST], f32r, tag="et", name="et_raw")
                raw_ets.append(et0)
                for hc in range(HC):
                    nc.sync.dma_start(
                        out=wk_sb[:, hc, 0:FT],
                        in_=wkT[hc * FT:(hc + 1) * FT, 0:FT])
                    nc.sync.dma_start(
                        out=et0[:, hc, :],
                        in_=encT[0, hc * FT:(hc + 1) * FT, 0:ST])
                for fc in range(1, HC):
                    load_wk_col(fc)
                for st in range(1, nraw):
                    raw_ets.append(load_et(0, st))
                load_wq()
            else:
                load_wq()
                emit_q()
                for hc in range(HC):
                    nc.sync.dma_start(out=wk_sb[:, hc, :],
                                      in_=wkT[hc * FT:(hc + 1) * FT, :])
            nc.sync.dma_start(out=v_sb[:], in_=vp[:])
            nc.sync.dma_start(out=mask_sb[:], in_=mb[:])

            args = (nc, tc, mybir, f32, f32r, Tanh, Exp, variant,
                    load_et, out, ep, kps, aps, stats,
                    wk_sb, v_sb, mask_sb, qT_sb, attn_sb, nm_sb)
            if loop_r > 1:
                with tc.For_i(0, loop_r, 1):
                    _body(*args, [], None, None)
            else:
                _body(*args, raw_ets, enr, emit_q)

    nc.compile()
    return nc


def _body(nc, tc, mybir, f32, f32r, Tanh, Exp, variant,
          load_et, out, ep, kps, aps, stats,
          wk_sb, v_sb, mask_sb, qT_sb, attn_sb, nm_sb, raw_ets, enr,
          emit_q):
    """Emit the main (b, st) tile loop.

    Startup: the first `nraw` seq-tiles of batch 0 run in "raw" mode —
    keys PSUM is drained to SBUF with a plain copy (no q dependency) so
    the PE streams matmuls from t=0 while Wq loads last; tanh runs
    in-place once q is ready and the deferred v-matvecs drain as gap
    fillers (at most 4 per flush point) inside later keys groups.

    Steady state: the v-matvec for a finished energy tile is emitted one
    PE group late (after matmul hc==4 of the next keys group) so the
    tanh latency never stalls the in-order PE queue.  A running
    per-batch max is maintained after each mask-add so the final
    softmax skips the full-row max reduce.
    """
    nraw = len(raw_ets)
    pending_v = []  # (ap_tile, energy_ap, fc, b, st)

    def softmax_row(b, negmax):
        r0 = 32 * b
        row = attn_sb[r0:r0 + 1, :]
        denom = stats.tile([1, 1], f32, tag="dn", bufs=2)
        nc.scalar.activation(out=row, in_=row, func=Exp,
                             bias=negmax[:], scale=1.0,
                             accum_out=denom[:])
        recip = stats.tile([1, 1], f32, tag="rc", bufs=2)
        nc.vector.reciprocal(out=recip[:], in_=denom[:])
        # normalize halves on DVE and ACT in parallel
        half = S // 2
        rowa = attn_sb[r0:r0 + 1, 0:half]
        rowb = attn_sb[r0:r0 + 1, half:S]
        nc.vector.tensor_scalar_mul(rowa, rowa, recip[:])
        nc.sync.dma_start(out=out[b:b + 1, 0:half], in_=rowa)
        nc.scalar.activation(out=rowb, in_=rowb,
                             func=mybir.ActivationFunctionType.Copy,
                             bias=0.0, scale=recip[:])
        nc.sync.dma_start(out=out[b:b + 1, half:S], in_=rowb)

    def flush(n):
        for _ in range(min(n, len(pending_v))):
            pap, pen, pfc, pb, pst = pending_v.pop(0)
            nc.tensor.matmul(
                pap[:], lhsT=v_sb[:, pfc:pfc + 1], rhs=pen,
                start=(pfc == 0), stop=(pfc == HC - 1))
            if pfc == HC - 1:
                r0 = 32 * pb
                sl = slice(pst * ST, (pst + 1) * ST)
                nc.vector.tensor_add(
                    out=attn_sb[r0:r0 + 1, sl],
                    in0=pap[:], in1=mask_sb[r0:r0 + 1, sl])
                cmax = stats.tile([1, 1], f32, tag="cm", bufs=2)
                nc.vector.tensor_reduce(
                    out=cmax[:], in_=attn_sb[r0:r0 + 1, sl],
                    axis=mybir.AxisListType.X, op=mybir.AluOpType.max)
                if pst < NST - 1:
                    nc.vector.tensor_scalar_max(
                        nm_sb[r0:r0 + 1, :], nm_sb[r0:r0 + 1, :], cmax[:])
                else:
                    # last chunk of the batch: fuse the final max-update
                    # with the negation the exp bias needs
                    negmax = stats.tile([1, 1], f32, tag="nm", bufs=2)
                    nc.vector.tensor_scalar(
                        negmax[:], cmax[:], nm_sb[r0:r0 + 1, :], -1.0,
                        mybir.AluOpType.max, mybir.AluOpType.mult)
                    if variant != "noSoftmax":
                        softmax_row(pb, negmax)

    def keys_group(et, fc):
        kp = kps.tile([FT, ST], f32, tag="kp")
        for hc in range(HC):
            nc.tensor.matmul(
                kp[:],
                lhsT=wk_sb[:, hc, fc * FT:(fc + 1) * FT],
                rhs=et[:, hc, :],
                start=(hc == 0), stop=(hc == HC - 1))
            if hc in (4, 7):
                flush(2)
        return kp

    # ---- raw startup tiles (b=0, st<nraw) ----
    state = {"q": False, "tanhed": 0}

    def raw_tanh_upto(limit):
        for st in range(state["tanhed"], limit):
            ap_ = aps.tile([1, ST], f32, tag="ap")
            for fc in range(HC):
                en = enr[:, st * HC + fc, :]
                nc.scalar.activation(out=en, in_=en, func=Tanh,
                                     bias=qT_sb[:, fc, 0:1], scale=1.0)
                pending_v.append((ap_, en, fc, 0, st))
        state["tanhed"] = limit

    for st in range(nraw):
        for fc in range(HC):
            kp = keys_group(raw_ets[st], fc)
            nc.vector.tensor_copy(out=enr[:, st * HC + fc, :], in_=kp[:])
        if st == 1 and emit_q is not None:
            # Wq has landed by now; running q here lets ACT chew the
            # deferred tanh backlog during the remaining raw keys
            emit_q()
            state["q"] = True
            raw_tanh_upto(2)
    if raw_ets:
        if not state["q"] and emit_q is not None:
            emit_q()
        raw_tanh_upto(nraw)

    # ---- steady-state tiles ----
    for b in range(BPC):
        for st in range(NST):
            if b == 0 and st < nraw:
                continue
            et = load_et(b, st)
            ap_ = aps.tile([1, ST], f32, tag="ap")
            for fc in range(HC):
                kp = keys_group(et, fc)
                en = ep.tile([FT, ST], f32r, tag="en")
                nc.scalar.activation(
                    out=en[:], in_=kp[:], func=Tanh,
                    bias=qT_sb[:, fc, b:b + 1], scale=1.0)
                pending_v.append((ap_, en[:], fc, b, st))
    flush(len(pending_v))

    if variant == "noSoftmax":
        rawt = stats.tile([BPC, S], f32)
        nc.vector.tensor_copy(out=rawt[:], in_=attn_sb[0:BPC, :])
        nc.sync.dma_start(out=out[:], in_=rawt[:])


def _get_nc():
    if "nc" not in _CACHE:
        _CACHE["nc"] = _build()
    return _CACHE["nc"]


def _prepare_in_maps(hidden, encoder_outputs, lengths, Wq, Wk, v):
    hidden = np.ascontiguousarray(np.asarray(hidden, dtype=np.float32))
    enc = np.asarray(encoder_outputs, dtype=np.float32)
    lengths = np.asarray(lengths).astype(np.int64)
    Wq = np.asarray(Wq, dtype=np.float32)
    Wk = np.asarray(Wk, dtype=np.float32)
    v = np.asarray(v, dtype=np.float32)

    encT = np.ascontiguousarray(enc.transpose(0, 2, 1))          # (B, H, S)
    hiddenT = np.ascontiguousarray(hidden.T)                     # (H, B)
    WkT = np.ascontiguousarray(Wk.T)                             # (H, H)
    WqT = np.ascontiguousarray(Wq.T)                             # (H, H)
    vp = np.ascontiguousarray(v.reshape(HC, FT).T)               # (128, 8)
    maskb = np.where(np.arange(S)[None, :] < lengths[:, None],
                     np.float32(0.0), np.float32(-1e30)).astype(np.float32)
    # pad to 128 partitions: batch b of the core sits at row 32*b
    maskp = np.zeros((NCORES, FT, S), dtype=np.float32)
    for c in range(NCORES):
        for b in range(BPC):
            maskp[c, 32 * b] = maskb[c * BPC + b]

    in_maps = []
    for c in range(NCORES):
        sl = slice(c * BPC, (c + 1) * BPC)
        in_maps.append({
            "encT": np.ascontiguousarray(encT[sl]),
            "hT": np.ascontiguousarray(hiddenT[:, sl]),
            "wkT": WkT,
            "wqT": WqT,
            "vp": vp,
            "mb": maskp[c],
        })
    return in_maps


def _run(in_maps, trace=False, **kw):
    from concourse.bass_utils import run_bass_kernel_spmd
    nc = _get_nc()
    res = run_bass_kernel_spmd(nc, in_maps, core_ids=list(range(NCORES)),
                               trace=trace, **kw)
    outs = np.concatenate([res.results[c]["out"] for c in range(NCORES)],
                          axis=0)
    return outs, res


def kernel(hidden, encoder_outputs, lengths, Wq, Wk, v):
    in_maps = _prepare_in_maps(hidden, encoder_outputs, lengths, Wq, Wk, v)
    outs, _ = _run(in_maps, trace=False)
    return outs



# revision 10
# speedup vs baseline: 1.5948x; 1.5948x over previous
"""Bahdanau-style additive attention on 8 TRN2 NeuronCores.

Reference computation (B=32, S=2048, H=1024):
    query  = hidden @ Wq.T                      # (B, H)
    keys   = enc @ Wk.T                         # (B, S, H)
    energy = tanh(query[:, None, :] + keys)     # (B, S, H)
    attn   = energy @ v                         # (B, S)
    out    = softmax(mask(attn, lengths))       # (B, S)

Sharding: data-parallel, 4 batches per core, no collectives.

Length-aware tile skipping: positions s >= lengths[b] contribute
exactly 0 to the output (softmax of -inf), so seq tiles that are fully
masked are never computed, DMA'd, or stored — the host fills those
output columns with zeros.  SPMD needs one program for all 8 cores, so
batches are sorted by length (descending) and dealt round-robin into 4
per-core slots; slot j's tile count is ceil(max-length-in-rank-octile-j
/ 512), baked into the program (rebuilt+cached per tile-count tuple).
The shortest slot runs last so the final softmax tail drains a short
row.

Per-core dataflow (all matmuls in fp32r mode — 1 cycle/row on the PE):
    - enc is fed pre-transposed (H, S) so the H contraction sits on SBUF
      partitions; tiles are [128h x 512s].
    - keys tile [128f, 512s] accumulates 8 h-chunk matmuls in PSUM.
    - ACT applies tanh with the per-partition query bias, PSUM -> SBUF.
    - PE contracts the energy tile with v ([128,1] stationary) into a
      [1, 512] PSUM accumulator over the 8 f-chunks.
    - DVE adds the (host-precomputed) length-mask bias and keeps a
      running per-batch max; the per-batch softmax tail is ACT exp with
      accumulate, then the normalize runs split across DVE and ACT.
"""

import sys

if "/opt/trn_rl_repo" not in sys.path:
    sys.path.insert(0, "/opt/trn_rl_repo")

import numpy as np

B, S, H = 32, 2048, 1024
NCORES = 8
BPC = B // NCORES  # batches per core
FT = 128           # partition tile (feature / h chunk)
HC = H // FT       # h chunks
ST = 512           # seq tile
NST = S // ST

_CACHE = {}


def _build(nsts=(NST,) * BPC, variant="full", loop_r=1):
    import concourse.bass as bass  # noqa: F401
    import concourse.tile as tile
    from concourse import bacc, mybir

    f32 = mybir.dt.float32
    f32r = mybir.dt.float32r
    Tanh = mybir.ActivationFunctionType.Tanh
    Exp = mybir.ActivationFunctionType.Exp

    nc = bacc.Bacc("TRN2", target_bir_lowering=False, debug=False,
                   num_devices=NCORES)

    encT = nc.dram_tensor("encT", [BPC, H, S], f32r, kind="ExternalInput").ap()
    hT = nc.dram_tensor("hT", [H, BPC], f32r, kind="ExternalInput").ap()
    wkT = nc.dram_tensor("wkT", [H, H], f32r, kind="ExternalInput").ap()
    wqT = nc.dram_tensor("wqT", [H, H], f32r, kind="ExternalInput").ap()
    vp = nc.dram_tensor("vp", [FT, HC], f32r, kind="ExternalInput").ap()
    # mask rows live at partition 32*b so engine APs stay 32-aligned
    mb = nc.dram_tensor("mb", [FT, S], f32, kind="ExternalInput").ap()
    out = nc.dram_tensor("out", [BPC, S], f32, kind="ExternalOutput").ap()

    nraw = min(3, nsts[0]) if loop_r == 1 else 0

    with tile.TileContext(nc) as tc:
        with (
            tc.tile_pool(name="singles", bufs=1) as singles,
            tc.tile_pool(name="encp", bufs=4) as encp,
            tc.tile_pool(name="energy", bufs=4) as ep,
            tc.tile_pool(name="kpsum", bufs=5, space="PSUM") as kps,
            tc.tile_pool(name="apsum", bufs=3, space="PSUM") as aps,
            tc.tile_pool(name="stats", bufs=1) as stats,
        ):
            wk_sb = singles.tile([FT, HC, H], f32r)
            wq_sb = singles.tile([FT, HC, H], f32r)
            ht_sb = singles.tile([FT, HC, BPC], f32r)
            v_sb = singles.tile([FT, HC], f32r)
            mask_sb = singles.tile([FT, S], f32)
            qT_sb = singles.tile([FT, HC, BPC], f32)
            attn_sb = singles.tile([FT, S], f32)
            nm_sb = singles.tile([FT, 1], f32)
            nc.vector.memset(attn_sb[:], 0.0)
            nc.vector.memset(nm_sb[:], -3.0e38)
            enr = (singles.tile([FT, nraw * HC, ST], f32r, name="enr")
                   if nraw else None)

            def emit_q():
                # qT[f, b] = sum_h WqT[h, f] * hiddenT[h, b].  All 8 fc
                # accumulation groups share one PSUM tile: `start` only
                # on the global first matmul (per-element has_written
                # handles first-touch for the other regions; nothing
                # reads the tile until after the global stop).
                qp = aps.tile([FT, HC, BPC], f32, tag="ap")
                for hc in range(HC):
                    for fc in range(HC):
                        nc.tensor.matmul(
                            qp[:, fc, :],
                            lhsT=wq_sb[:, hc, fc * FT:(fc + 1) * FT],
                            rhs=ht_sb[:, hc, :],
                            start=(hc == 0 and fc == 0),
                            stop=(hc == HC - 1 and fc == HC - 1),
                            skip_group_check=True)
                nc.vector.tensor_copy(out=qT_sb[:], in_=qp[:])

            def load_wq():
                for hc in range(HC):
                    nc.sync.dma_start(out=wq_sb[:, hc, :],
                                      in_=wqT[hc * FT:(hc + 1) * FT, :])
                    nc.sync.dma_start(out=ht_sb[:, hc, :],
                                      in_=hT[hc * FT:(hc + 1) * FT, :])

            def load_et(b, st):
                et = encp.tile([FT, HC, ST], f32r, tag="et", name="et")
                for hc in range(HC):
                    nc.sync.dma_start(
                        out=et[:, hc, :],
                        in_=encT[b, hc * FT:(hc + 1) * FT,
                                 st * ST:(st + 1) * ST])
                return et

            def load_wk_col(fc):
                nc.sync.dma_start(
                    out=wk_sb[:, :, fc * FT:(fc + 1) * FT],
                    in_=wkT[:, fc * FT:(fc + 1) * FT].rearrange(
                        "(hc p) f -> p hc f", p=FT))

            raw_ets = []
            if nraw:
                # DMA order: Wk column 0 in per-hc 64KB blocks
                # interleaved with the et0 chunks (the first keys matmul
                # needs only block hc=0, so the PE starts at ~1.5us),
                # remaining Wk columns (keys group fc only needs col
                # fc), other raw tiles, Wq LAST — the PE has ~40us of
                # keys work queued by the time q runs.
                et0 = encp.tile([FT, HC, ST], f32r, tag="et", name="et_raw")
                raw_ets.append(et0)
                for hc in range(HC):
                    nc.sync.dma_start(
                        out=wk_sb[:, hc, 0:FT],
                        in_=wkT[hc * FT:(hc + 1) * FT, 0:FT])
                    nc.sync.dma_start(
                        out=et0[:, hc, :],
                        in_=encT[0, hc * FT:(hc + 1) * FT, 0:ST])
                for fc in range(1, HC):
                    load_wk_col(fc)
                for st in range(1, nraw):
                    raw_ets.append(load_et(0, st))
                load_wq()
            else:
                load_wq()
                emit_q()
                for hc in range(HC):
                    nc.sync.dma_start(out=wk_sb[:, hc, :],
                                      in_=wkT[hc * FT:(hc + 1) * FT, :])
            nc.sync.dma_start(out=v_sb[:], in_=vp[:])
            nc.sync.dma_start(out=mask_sb[:], in_=mb[:])

            args = (nc, tc, mybir, f32, f32r, Tanh, Exp, variant, nsts,
                    load_et, out, ep, kps, aps, stats,
                    wk_sb, v_sb, mask_sb, qT_sb, attn_sb, nm_sb)
            if loop_r > 1:
                with tc.For_i(0, loop_r, 1):
                    _body(*args, [], None, None)
            else:
                _body(*args, raw_ets, enr, emit_q)

    nc.compile()
    return nc


def _body(nc, tc, mybir, f32, f32r, Tanh, Exp, variant, nsts,
          load_et, out, ep, kps, aps, stats,
          wk_sb, v_sb, mask_sb, qT_sb, attn_sb, nm_sb, raw_ets, enr,
          emit_q):
    """Emit the main (b, st) tile loop.

    Startup: the first `nraw` seq-tiles of batch 0 run in "raw" mode —
    keys PSUM is drained to SBUF with a plain copy (no q dependency) so
    the PE streams matmuls from t=0 while Wq loads last; tanh runs
    in-place once q is ready and the deferred v-matvecs drain as gap
    fillers (at most 4 per flush point) inside later keys groups.

    Steady state: the v-matvec for a finished energy tile is emitted one
    PE group late (after matmul hc==4 of the next keys group) so the
    tanh latency never stalls the in-order PE queue.  A running
    per-batch max is maintained after each mask-add so the final
    softmax skips the full-row max reduce.
    """
    nraw = len(raw_ets)
    pending_v = []  # (ap_tile, energy_ap, fc, b, st)

    def softmax_row(b, negmax):
        L = nsts[b] * ST
        r0 = 32 * b
        row = attn_sb[r0:r0 + 1, 0:L]
        denom = stats.tile([1, 1], f32, tag="dn", bufs=2)
        nc.scalar.activation(out=row, in_=row, func=Exp,
                             bias=negmax[:], scale=1.0,
                             accum_out=denom[:])
        recip = stats.tile([1, 1], f32, tag="rc", bufs=2)
        nc.vector.reciprocal(out=recip[:], in_=denom[:])
        # normalize halves on DVE and ACT in parallel
        half = L // 2
        rowa = attn_sb[r0:r0 + 1, 0:half]
        rowb = attn_sb[r0:r0 + 1, half:L]
        nc.vector.tensor_scalar_mul(rowa, rowa, recip[:])
        nc.sync.dma_start(out=out[b:b + 1, 0:half], in_=rowa)
        nc.scalar.activation(out=rowb, in_=rowb,
                             func=mybir.ActivationFunctionType.Copy,
                             bias=0.0, scale=recip[:])
        nc.sync.dma_start(out=out[b:b + 1, half:L], in_=rowb)

    def flush(n):
        for _ in range(min(n, len(pending_v))):
            pap, pen, pfc, pb, pst = pending_v.pop(0)
            nc.tensor.matmul(
                pap[:], lhsT=v_sb[:, pfc:pfc + 1], rhs=pen,
                start=(pfc == 0), stop=(pfc == HC - 1))
            if pfc == HC - 1:
                r0 = 32 * pb
                sl = slice(pst * ST, (pst + 1) * ST)
                nc.vector.tensor_add(
                    out=attn_sb[r0:r0 + 1, sl],
                    in0=pap[:], in1=mask_sb[r0:r0 + 1, sl])
                cmax = stats.tile([1, 1], f32, tag="cm", bufs=2)
                nc.vector.tensor_reduce(
                    out=cmax[:], in_=attn_sb[r0:r0 + 1, sl],
                    axis=mybir.AxisListType.X, op=mybir.AluOpType.max)
                if pst < nsts[pb] - 1:
                    nc.vector.tensor_scalar_max(
                        nm_sb[r0:r0 + 1, :], nm_sb[r0:r0 + 1, :], cmax[:])
                else:
                    # last chunk of the batch: fuse the final max-update
                    # with the negation the exp bias needs
                    negmax = stats.tile([1, 1], f32, tag="nm", bufs=2)
                    nc.vector.tensor_scalar(
                        negmax[:], cmax[:], nm_sb[r0:r0 + 1, :], -1.0,
                        mybir.AluOpType.max, mybir.AluOpType.mult)
                    if variant != "noSoftmax":
                        softmax_row(pb, negmax)

    def keys_group(et, fc):
        kp = kps.tile([FT, ST], f32, tag="kp")
        for hc in range(HC):
            nc.tensor.matmul(
                kp[:],
                lhsT=wk_sb[:, hc, fc * FT:(fc + 1) * FT],
                rhs=et[:, hc, :],
                start=(hc == 0), stop=(hc == HC - 1))
            if hc in (4, 7):
                flush(2)
        return kp

    # ---- raw startup tiles (b=0, st<nraw) ----
    state = {"q": False, "tanhed": 0}

    def raw_tanh_upto(limit):
        for st in range(state["tanhed"], limit):
            ap_ = aps.tile([1, ST], f32, tag="ap")
            for fc in range(HC):
                en = enr[:, st * HC + fc, :]
                nc.scalar.activation(out=en, in_=en, func=Tanh,
                                     bias=qT_sb[:, fc, 0:1], scale=1.0)
                pending_v.append((ap_, en, fc, 0, st))
        state["tanhed"] = limit

    for st in range(nraw):
        for fc in range(HC):
            kp = keys_group(raw_ets[st], fc)
            nc.vector.tensor_copy(out=enr[:, st * HC + fc, :], in_=kp[:])
        if st == 1 and emit_q is not None:
            # Wq has landed by now; running q here lets ACT chew the
            # deferred tanh backlog during the remaining raw keys
            emit_q()
            state["q"] = True
            raw_tanh_upto(2)
    if raw_ets:
        if not state["q"] and emit_q is not None:
            emit_q()
        raw_tanh_upto(nraw)

    # ---- steady-state tiles ----
    for b in range(BPC):
        for st in range(nsts[b]):
            if b == 0 and st < nraw:
                continue
            et = load_et(b, st)
            ap_ = aps.tile([1, ST], f32, tag="ap")
            for fc in range(HC):
                kp = keys_group(et, fc)
                en = ep.tile([FT, ST], f32r, tag="en")
                nc.scalar.activation(
                    out=en[:], in_=kp[:], func=Tanh,
                    bias=qT_sb[:, fc, b:b + 1], scale=1.0)
                pending_v.append((ap_, en[:], fc, b, st))
    flush(len(pending_v))

    if variant == "noSoftmax":
        rawt = stats.tile([BPC, S], f32)
        nc.vector.tensor_copy(out=rawt[:], in_=attn_sb[0:BPC, :])
        nc.sync.dma_start(out=out[:], in_=rawt[:])


def _get_nc(nsts):
    key = ("nc", tuple(nsts))
    if key not in _CACHE:
        _CACHE[key] = _build(tuple(nsts))
    return _CACHE[key]


def _plan(lengths):
    """Deal batches (sorted by length, descending) round-robin into the
    4 per-core slots; slot tile counts come from each rank-octile max."""
    lengths = np.asarray(lengths).astype(np.int64)
    order = np.argsort(-lengths, kind="stable")
    slots = order.reshape(BPC, NCORES)        # slots[j, c] -> batch index
    nsts = tuple(int(np.ceil(lengths[slots[j]].max() / ST))
                 for j in range(BPC))
    return slots, nsts


def _prepare_in_maps(hidden, encoder_outputs, lengths, Wq, Wk, v):
    hidden = np.ascontiguousarray(np.asarray(hidden, dtype=np.float32))
    enc = np.asarray(encoder_outputs, dtype=np.float32)
    lengths = np.asarray(lengths).astype(np.int64)
    Wq = np.asarray(Wq, dtype=np.float32)
    Wk = np.asarray(Wk, dtype=np.float32)
    v = np.asarray(v, dtype=np.float32)

    slots, nsts = _plan(lengths)

    hiddenT = np.ascontiguousarray(hidden.T)                     # (H, B)
    WkT = np.ascontiguousarray(Wk.T)                             # (H, H)
    WqT = np.ascontiguousarray(Wq.T)                             # (H, H)
    vp = np.ascontiguousarray(v.reshape(HC, FT).T)               # (128, 8)
    maskb = np.where(np.arange(S)[None, :] < lengths[:, None],
                     np.float32(0.0), np.float32(-1e30)).astype(np.float32)
    # pad to 128 partitions: slot j of the core sits at row 32*j
    maskp = np.zeros((NCORES, FT, S), dtype=np.float32)
    for c in range(NCORES):
        for j in range(BPC):
            maskp[c, 32 * j] = maskb[slots[j, c]]

    in_maps = []
    for c in range(NCORES):
        bs = slots[:, c]
        in_maps.append({
            "encT": np.ascontiguousarray(enc[bs].transpose(0, 2, 1)),
            "hT": np.ascontiguousarray(hiddenT[:, bs]),
            "wkT": WkT,
            "wqT": WqT,
            "vp": vp,
            "mb": maskp[c],
        })
    return in_maps, slots, nsts


def _run(in_maps, nsts, trace=False, **kw):
    from concourse.bass_utils import run_bass_kernel_spmd
    nc = _get_nc(nsts)
    res = run_bass_kernel_spmd(nc, in_maps, core_ids=list(range(NCORES)),
                               trace=trace, **kw)
    return res


def _assemble(res, slots, nsts):
    out = np.zeros((B, S), dtype=np.float32)
    for c in range(NCORES):
        for j in range(BPC):
            L = nsts[j] * ST
            out[slots[j, c], :L] = res.results[c]["out"][j, :L]
    return out


def kernel(hidden, encoder_outputs, lengths, Wq, Wk, v):
    in_maps, slots, nsts = _prepare_in_maps(hidden, encoder_outputs,
                                            lengths, Wq, Wk, v)
    res = _run(in_maps, nsts, trace=False)
    return _assemble(res, slots, nsts)

